# revision 14
# baseline (speedup 1.0000x reference)
"""2-layer GCN (GCNConv x2, symmetric norm, self-loops) on 8 Trainium2 NeuronCores.

Strategy (graph/data parallel):
  - Nodes are partitioned contiguously across 8 cores (6250/core, padded to
    6272 = 49*128). Within a core, nodes are permuted into 49 blocks of 128
    with greedy in-degree balancing so every block has the same padded edge
    slot count (compile-time constant).
  - norm = dinv[src]*dinv[dst] is factored: dinv[src] is folded into x on
    the host (x' = dinv*x so h' = x'@W1 = dinv*h), dinv[dst] is applied in
    the per-block epilogue. Self-loop edges are dropped from the gather
    slots and handled by an identity matmul against the locally-kept h'/g'
    blocks (dinv_d^2*h_d = dinv_d * h'_d).
  - Layer 1 dense transform h' = x' @ W1 is row-sharded: each core
    multiplies its [6272, 4096] x'-slice (fed pre-transposed bf16) against
    replicated W1.
  - h' is AllGather'd to a full [50176, 256] per-core DRAM table; each core
    aggregates its own dst nodes: dma_gather fetches h'[src] rows and a
    0/1 one-hot selection matrix M (built on-device with ONE batched
    is_equal per superblock using stride-0 broadcast APs) scatter-adds them
    on the TensorEngine with PSUM accumulation; M is the stationary
    operand so each chunk costs one LDWEIGHTS + one 256-wide stream and
    the result lands node-major [dst, feat].
  - epilogue per block: (+ self-loop identity matmul), *dinv_d, +b1, ReLU
    (on ACT), PE-transpose, @W2, *dinv_d -> g'. g' is AllGather'd and the
    same gather/one-hot aggregation + bias produces the output.
  - dma_gather indices are int16, so the gather table is split at row
    32640 into unequal lo/hi parts (the largest block-aligned boundary an
    int16 can address); edges are slotted by src part and the weighted
    block packer normalizes the unequal half loads (c_lo=11 + c_hi=6
    chunks/block vs 9+9 for an even split).

kernel(**inputs) takes full unsharded inputs, returns the full [50000, 128]
output. Self-contained: no sibling imports; /opt/trn_rl_repo provides bass.
"""

import math
import sys

import numpy as np

sys.path.insert(0, "/opt/trn_rl_repo")

import concourse.bass as bass  # noqa: E402
import concourse.mybir as mybir  # noqa: E402
import concourse.tile as tile  # noqa: E402
from concourse import bacc  # noqa: E402

P = 128
NCORES = 8
SB = 2  # blocks per gather superblock
GA = 8  # blocks per phase-A psum group

F32 = mybir.dt.float32
BF16 = mybir.dt.bfloat16
I16 = mybir.dt.int16


# ---------------------------------------------------------------------------
# host-side preprocessing
# ---------------------------------------------------------------------------

def _superblocks(nb):
    return [tuple(range(s, min(s + SB, nb))) for s in range(0, nb, SB)]


def _swizzle_idx(idx):
    """gather idx j -> [j%16, j//16], replicated across the 8 groups of 16."""
    n = idx.shape[0]
    a = np.zeros((16, n // 16), np.int16)
    a[np.arange(n) % 16, np.arange(n) // 16] = idx.astype(np.int16)
    return np.tile(a, (8, 1))


def _pack_blocks_2d(dlo, dhi, nb, w_lo=1.0, w_hi=1.0):
    """Greedy pack minimizing the weighted max per-half block load (both
    halves are padded to a global per-half chunk count, so the max is what
    costs; weights normalize unequal half means)."""
    n = dlo.shape[0]
    order = np.argsort(-(dlo + dhi), kind="stable")
    cur_lo = np.zeros(nb)
    cur_hi = np.zeros(nb)
    counts = np.zeros(nb, np.int64)
    pos = np.empty(n, np.int64)
    for i in order:
        score = np.maximum((cur_lo + dlo[i]) * w_lo, (cur_hi + dhi[i]) * w_hi)
        score[counts >= P] = np.inf
        b = int(np.argmin(score))
        pos[i] = b * P + counts[b]
        counts[b] += 1
        cur_lo[b] += dlo[i]
        cur_hi[b] += dhi[i]
    return pos


def _prep(x, edge_index, W1, b1, W2, b2):
    import ml_dtypes

    N, F_in = x.shape
    F_h = W1.shape[1]
    F_out = W2.shape[1]
    assert N % NCORES == 0 and F_in % P == 0 and F_h == 2 * P and F_out == P
    npc_raw = N // NCORES
    nb = math.ceil(npc_raw / P)
    npc = nb * P
    ntot = NCORES * npc
    # lo/hi gather-table split: the boundary is the largest row count an
    # int16 index can address; the halves are unequal so the weighted
    # packer normalizes them.
    thalf = min(2 ** 15 - 1, ntot - P)
    thalf -= thalf % P

    # self-loops are NOT slotted as edges (identity-matmul path); they do
    # count toward deg.
    src = np.asarray(edge_index[0]).astype(np.int64)
    dst = np.asarray(edge_index[1]).astype(np.int64)
    deg = (np.bincount(np.concatenate([dst, np.arange(N)]), minlength=N)
           .astype(np.float64))
    dinv = np.where(deg > 0, 1.0 / np.sqrt(deg), 0.0)

    core_dst = dst // npc_raw

    # row id of src depends only on (core, pos); pos only permutes within
    # a core, so the lo/hi class of an edge is fixed by the row range the
    # src CORE occupies -- except for the boundary core, whose nodes can
    # fall on either side depending on pos. Classify provisionally by the
    # core's row base using pos-independent bounds, then pack, then
    # finalize with actual pos.
    hi_src = ((src // npc_raw) * npc) + npc // 2 >= thalf  # provisional

    # per-node in-degree by half, for block balancing
    d_lo = np.bincount(dst[~hi_src], minlength=N)
    d_hi = np.bincount(dst[hi_src], minlength=N)
    frac_lo = max(d_lo.sum(), 1) / max(d_lo.sum() + d_hi.sum(), 1)
    w_lo = 1.0 / max(frac_lo, 1e-6)
    w_hi = 1.0 / max(1.0 - frac_lo, 1e-6)

    pos = np.empty(N, np.int64)
    for c in range(NCORES):
        lo = c * npc_raw
        nodes = np.arange(lo, lo + npc_raw)
        pos[nodes] = _pack_blocks_2d(d_lo[nodes].astype(np.float64),
                                     d_hi[nodes].astype(np.float64), nb,
                                     w_lo, w_hi)
    rowof = (np.arange(N) // npc_raw) * npc + pos
    hi_src = rowof[src] >= thalf  # final, pos-exact

    # per (core, block, half) edge counts -> global chunk constants
    blk_of_dst = pos[dst] // P
    cnt = np.zeros((NCORES, nb, 2), np.int64)
    np.add.at(cnt, (core_dst, blk_of_dst, hi_src.astype(np.int64)), 1)
    c_lo = int(math.ceil(cnt[:, :, 0].max() / P))
    c_hi = int(math.ceil(cnt[:, :, 1].max() / P))
    c_tot = c_lo + c_hi

    sbs = _superblocks(nb)
    nchunks = nb * c_tot

    cores = []
    for c in range(NCORES):
        mask = core_dst == c
        e_src_row = rowof[src[mask]]
        e_hi = hi_src[mask]
        e_blk = blk_of_dst[mask]
        e_dl = (pos[dst[mask]] % P).astype(np.float32)

        # slot arrays per half: block-major, padded to C_half*128 per block
        idx_flat = {0: np.zeros(nb * c_lo * P, np.int64),
                    1: np.zeros(nb * c_hi * P, np.int64)}
        # dst-local slot value per (block, chunk, slot); 255 = inert pad
        dl_by_chunk = np.full((nb, c_tot, P), 255.0, np.float32)
        for h, c_half, base in ((0, c_lo, 0), (1, c_hi, c_lo)):
            sel = e_hi == (h == 1)
            sr = e_src_row[sel] - (thalf if h == 1 else 0)
            bl = e_blk[sel]
            order = np.argsort(bl, kind="stable")
            sr, bl = sr[order], bl[order]
            dl_s = e_dl[sel][order]
            start = np.searchsorted(bl, np.arange(nb))
            end = np.searchsorted(bl, np.arange(nb) + 1)
            for b in range(nb):
                k = end[b] - start[b]
                assert k <= c_half * P
                sl = slice(start[b], end[b])
                idx_flat[h][b * c_half * P: b * c_half * P + k] = sr[sl]
                flat_dl = np.full(c_half * P, 255.0, np.float32)
                flat_dl[:k] = dl_s[sl]
                dl_by_chunk[b, base: base + c_half] = flat_dl.reshape(c_half, P)

        assert idx_flat[0].min() >= 0 and idx_flat[0].max() < thalf
        assert idx_flat[1].min() >= 0 and idx_flat[1].max() < thalf

        # mdst in device chunk order: per superblock, [blocks x lo] then
        # [blocks x hi]; within each, chunk-major per block. Layout:
        # [P slots, nchunks] (slot value = dst-local of that slot).
        md = []
        for blocks in sbs:
            for b in blocks:
                md.append(dl_by_chunk[b, :c_lo])
            for b in blocks:
                md.append(dl_by_chunk[b, c_lo:])
        mdst = np.concatenate(md).reshape(nchunks, P).T.copy()  # [P, nchunks]

        # x slice, permuted, pre-scaled by dinv[node], transposed:
        # xt[f, pos] = dinv[node] * x[node, f]
        nodes = np.arange(c * npc_raw, (c + 1) * npc_raw)
        xp = np.zeros((npc, F_in), np.float32)
        xp[pos[nodes]] = np.asarray(x[nodes], np.float32) * \
            dinv[nodes, None].astype(np.float32)
        xt = np.ascontiguousarray(xp.T.astype(np.float32))
        xt = xt.astype(ml_dtypes.bfloat16)

        # dinv per dst slot, block-major: dinvd[p, b]
        dv = np.zeros(npc, np.float32)
        dv[pos[nodes]] = dinv[nodes].astype(np.float32)
        dinvd = dv.reshape(nb, P).T.copy()  # [P, nb]

        cores.append({
            "xt": xt,
            "idx_lo": _swizzle_idx(idx_flat[0]),
            "idx_hi": _swizzle_idx(idx_flat[1]),
            "mdst": ml_dtypes.bfloat16(mdst),
            "dinvd": dinvd,
        })

    shared = {
        "w1": ml_dtypes.bfloat16(np.asarray(W1, np.float32)),
        "w2": ml_dtypes.bfloat16(np.asarray(W2, np.float32)),
        "b1b": ml_dtypes.bfloat16(
            np.tile(np.asarray(b1, np.float32)[None, :], (P, 1))),
        "b2b": np.tile(np.asarray(b2, np.float32)[None, :], (P, 1)),
        "iota": ml_dtypes.bfloat16(
            np.tile(np.arange(P, dtype=np.float32)[None, :], (P, 1))),
        "ident": ml_dtypes.bfloat16(np.eye(P, dtype=np.float32)),
    }
    cfg = dict(N=N, F_in=F_in, F_h=F_h, F_out=F_out, npc_raw=npc_raw, nb=nb,
               npc=npc, ntot=ntot, thalf=thalf, c_lo=c_lo, c_hi=c_hi,
               c_tot=c_tot, nchunks=nchunks, pos=pos)
    return cfg, cores, shared


# ---------------------------------------------------------------------------
# device kernel
# ---------------------------------------------------------------------------

def _build_nc(cfg):
    F_in, F_h, F_out = cfg["F_in"], cfg["F_h"], cfg["F_out"]
    nb, npc, ntot, thalf = cfg["nb"], cfg["npc"], cfg["ntot"], cfg["thalf"]
    c_lo, c_hi, c_tot = cfg["c_lo"], cfg["c_hi"], cfg["c_tot"]
    nchunks = cfg["nchunks"]
    kt = F_in // P
    sbs = _superblocks(nb)
    rg = [list(range(NCORES))]

    nc = bacc.Bacc(None, num_devices=NCORES, num_swdge_queues=4)

    xt_d = nc.declare_dram_parameter("xt", [F_in, npc], BF16, isOutput=False)
    w1_d = nc.declare_dram_parameter("w1", [F_in, F_h], BF16, isOutput=False)
    w2_d = nc.declare_dram_parameter("w2", [F_h, F_out], BF16, isOutput=False)
    b1_d = nc.declare_dram_parameter("b1b", [P, F_h], BF16, isOutput=False)
    b2_d = nc.declare_dram_parameter("b2b", [P, F_out], F32, isOutput=False)
    iota_d = nc.declare_dram_parameter("iota", [P, P], BF16, isOutput=False)
    id_d = nc.declare_dram_parameter("ident", [P, P], BF16, isOutput=False)
    dv_d = nc.declare_dram_parameter("dinvd", [P, nb], F32, isOutput=False)
    ilo_d = nc.declare_dram_parameter("idx_lo", [P, nb * c_lo * 8], I16, isOutput=False)
    ihi_d = nc.declare_dram_parameter("idx_hi", [P, nb * c_hi * 8], I16, isOutput=False)
    mdst_d = nc.declare_dram_parameter("mdst", [P, nchunks], BF16, isOutput=False)
    out_d = nc.declare_dram_parameter("out", [npc, F_out], F32, isOutput=True)

    with tile.TileContext(nc) as tc:
        with (
            tc.tile_pool(name="const", bufs=1) as const,
            tc.tile_pool(name="work", bufs=1) as work,
            tc.tile_pool(name="dram", bufs=1, space="DRAM") as dram,
        ):
            h_own = dram.tile([npc, F_h], BF16)
            h_full = dram.tile([ntot, F_h], BF16, addr_space="Shared")
            g_own = dram.tile([npc, F_out], BF16)
            g_full = dram.tile([ntot, F_out], BF16, addr_space="Shared")

            w1_t = const.tile([P, kt, F_h], BF16)
            w2_t = const.tile([P, 2, F_out], BF16)
            b1_t = const.tile([P, F_h], BF16)
            b2_t = const.tile([P, F_out], F32)
            iota_t = const.tile([P, P], BF16)
            id_t = const.tile([P, P], BF16)
            dv_t = const.tile([P, nb], F32)
            ilo_t = const.tile([P, nb * c_lo * 8], I16)
            ihi_t = const.tile([P, nb * c_hi * 8], I16)
            mdst_t = const.tile([P, nchunks], BF16)
            h_keep = const.tile([P, nb, F_h], BF16)
            g_keep = const.tile([P, nb, F_out], BF16)

            w1_r = w1_d[:].rearrange("(a p) o -> p a o", p=P)
            nc.sync.dma_start(w1_t[:, 0:1, :], w1_r[:, 0:1, :])
            nc.sync.dma_start(w1_t[:, 1:, :], w1_r[:, 1:, :])
            nc.scalar.dma_start(w2_t[:], w2_d[:].rearrange("(h p) o -> p h o", p=P))
            nc.scalar.dma_start(b1_t[:], b1_d[:])
            nc.scalar.dma_start(b2_t[:], b2_d[:])
            nc.scalar.dma_start(iota_t[:], iota_d[:])
            nc.scalar.dma_start(id_t[:], id_d[:])
            nc.scalar.dma_start(dv_t[:], dv_d[:])
            nc.scalar.dma_start(ilo_t[:], ilo_d[:])
            nc.scalar.dma_start(ihi_t[:], ihi_d[:])
            nc.scalar.dma_start(mdst_t[:], mdst_d[:])

            # PE warm-up: ~40 throwaway matmuls so the HAM clock-gate is
            # released before the real phase-A stream begins.
            warm_pool = tc.tile_pool(name="psumW", bufs=1, space="PSUM")
            pw = warm_pool.__enter__()
            pwt = pw.tile([P, P], F32, tag="pw", bufs=1, space="PSUM")
            for _ in range(40):
                nc.tensor.matmul(pwt[:], lhsT=iota_t[:], rhs=iota_t[:],
                                 start=True, stop=True)
            warm_pool.__exit__(None, None, None)

            # ---- phase A: h'_own = x' @ W1 (rows = this core's nodes) ----
            # Blocks are processed in groups of GA=8 so each xt DMA reads
            # 8*128 contiguous columns (2 KB/partition); DMA issue alternates
            # between the two HWDGE rings (sync / scalar engines).
            xt_r = xt_d[:].rearrange("(a p) n -> p a n", p=P)
            h_own_r = h_own[:].rearrange("(b p) f -> p b f", p=P)
            g_own_r = g_own[:].rearrange("(b p) f -> p b f", p=P)
            out_r = out_d[:].rearrange("(b p) f -> p b f", p=P)
            psumA = tc.tile_pool(name="psumA", bufs=1, space="PSUM")
            psum = psumA.__enter__()
            dma_i = [0]

            def dma_alt(dst, src):
                eng = nc.sync if dma_i[0] % 2 == 0 else nc.scalar
                eng.dma_start(dst, src)
                dma_i[0] += 1

            for g0 in range(0, nb, GA):
                gb = list(range(g0, min(g0 + GA, nb)))
                phs = [psum.tile([P, F_h], F32, tag="ph", bufs=GA,
                                 space="PSUM", name=f"ph{g0}_{i}")
                       for i in range(len(gb))]
                for a in range(kt):
                    xt_t = work.tile([P, len(gb) * P], BF16, tag="xt", bufs=6)
                    dma_alt(xt_t[:], xt_r[:, a, g0 * P:g0 * P + len(gb) * P])
                    for i in range(len(gb)):
                        nc.tensor.matmul(phs[i][:],
                                         lhsT=xt_t[:, i * P:(i + 1) * P],
                                         rhs=w1_t[:, a, :],
                                         start=(a == 0), stop=(a == kt - 1))
                for i, b in enumerate(gb):
                    nc.vector.tensor_copy(h_keep[:, b, :], phs[i][:])
                dma_alt(h_own_r[:, g0:g0 + len(gb), :],
                        h_keep[:, g0:g0 + len(gb), :])

            psumA.__exit__(None, None, None)
            nc.gpsimd.collective_compute(
                "AllGather", mybir.AluOpType.bypass, replica_groups=rg,
                ins=[h_own[:]], outs=[h_full[:]],
            )
            psumC = tc.tile_pool(name="psumC", bufs=1, space="PSUM")
            psum = psumC.__enter__()

            # HW cap: >768 idxs in one dma_gather crashes the exec unit
            # (~128 in-flight descs per Q7 core); split and spread over the
            # 4 SWDGE queues.
            GMAX = 6  # chunks (of 128 idxs) per dma_gather op
            qn = [0]

            def gathers(dst, c0, nch, table, idx_t, col0, elem):
                for s in range(0, nch, GMAX):
                    k = min(GMAX, nch - s)
                    nc.gpsimd.dma_gather(
                        out_ap=dst[:, c0 + s:c0 + s + k, :], in_ap=table,
                        idxs_ap=idx_t[:, col0 + s * 8:col0 + (s + k) * 8],
                        num_idxs=k * P, num_idxs_reg=k * P, elem_size=elem,
                        queue_num=qn[0] % 4)
                    qn[0] += 1

            def m_build(gc0, nch):
                m = work.tile([P, nch, P], BF16, tag="m", bufs=3)
                nc.vector.tensor_tensor(
                    out=m[:],
                    in0=iota_t[:].unsqueeze(1).to_broadcast([P, nch, P]),
                    in1=mdst_t[:, gc0:gc0 + nch].unsqueeze(2)
                        .to_broadcast([P, nch, P]),
                    op=mybir.AluOpType.is_equal)
                return m

            # ---- phase C: aggregate layer 1, epilogue, transform by W2 ----
            gc_base = 0
            for blocks in sbs:
                k = len(blocks)
                msg = work.tile([P, k * c_tot, F_h], BF16, tag="msg", bufs=4)
                b0 = blocks[0]
                gathers(msg, 0, k * c_lo, h_full[0:thalf, :],
                        ilo_t, b0 * c_lo * 8, F_h)
                gathers(msg, k * c_lo, k * c_hi, h_full[thalf:ntot, :],
                        ihi_t, b0 * c_hi * 8, F_h)
                m = m_build(gc_base, k * c_tot)
                for bi, b in enumerate(blocks):
                    pa = psum.tile([P, F_h], F32, tag="pa", bufs=2, space="PSUM")
                    chunks = ([bi * c_lo + j for j in range(c_lo)] +
                              [k * c_lo + bi * c_hi + j for j in range(c_hi)])
                    for ci, c in enumerate(chunks):
                        nc.tensor.matmul(pa[:], lhsT=m[:, c, :],
                                         rhs=msg[:, c, :],
                                         start=(ci == 0), stop=False)
                    # self-loop: += I * h'_block (dinv_d^2 h = dinv_d h')
                    nc.tensor.matmul(pa[:], lhsT=id_t[:], rhs=h_keep[:, b, :],
                                     start=False, stop=True)
                    t2 = work.tile([P, F_h], BF16, tag="t2", bufs=2)
                    nc.vector.tensor_scalar(
                        out=t2[:], in0=pa[:], scalar1=dv_t[:, b:b + 1],
                        scalar2=None, op0=mybir.AluOpType.mult)
                    t3 = work.tile([P, F_h], BF16, tag="t3", bufs=2)
                    nc.vector.tensor_tensor(out=t3[:], in0=t2[:], in1=b1_t[:],
                                            op=mybir.AluOpType.add)
                    rh = work.tile([P, F_h], BF16, tag="rh", bufs=2)
                    nc.scalar.activation(rh[:], t3[:],
                                         mybir.ActivationFunctionType.Relu)
                    pg = psum.tile([P, F_out], F32, tag="pg", bufs=2,
                                   space="PSUM")
                    for half in range(2):
                        pt = psum.tile([P, P], BF16, tag="pt", bufs=2,
                                       space="PSUM")
                        nc.tensor.transpose(
                            pt[:], rh[:, half * P:(half + 1) * P], id_t[:])
                        rt = work.tile([P, P], BF16, tag="rt", bufs=2)
                        nc.vector.tensor_copy(rt[:], pt[:])
                        nc.tensor.matmul(pg[:], lhsT=rt[:],
                                         rhs=w2_t[:, half, :],
                                         start=(half == 0), stop=(half == 1))
                    # g' = dinv_d * g
                    nc.vector.tensor_scalar(
                        out=g_keep[:, b, :], in0=pg[:],
                        scalar1=dv_t[:, b:b + 1], scalar2=None,
                        op0=mybir.AluOpType.mult)
                dma_alt(g_own_r[:, b0:b0 + k, :], g_keep[:, b0:b0 + k, :])
                gc_base += k * c_tot

            nc.gpsimd.collective_compute(
                "AllGather", mybir.AluOpType.bypass, replica_groups=rg,
                ins=[g_own[:]], outs=[g_full[:]],
            )

            # ---- phase E: aggregate layer 2, add bias, write out ----
            gc_base = 0
            for blocks in sbs:
                k = len(blocks)
                msg2 = work.tile([P, k * c_tot, F_out], BF16, tag="msg", bufs=4)
                b0 = blocks[0]
                gathers(msg2, 0, k * c_lo, g_full[0:thalf, :],
                        ilo_t, b0 * c_lo * 8, F_out)
                gathers(msg2, k * c_lo, k * c_hi, g_full[thalf:ntot, :],
                        ihi_t, b0 * c_hi * 8, F_out)
                m = m_build(gc_base, k * c_tot)
                for bi, b in enumerate(blocks):
                    po = psum.tile([P, F_out], F32, tag="pa", bufs=2,
                                   space="PSUM")
                    chunks = ([bi * c_lo + j for j in range(c_lo)] +
                              [k * c_lo + bi * c_hi + j for j in range(c_hi)])
                    for ci, c in enumerate(chunks):
                        nc.tensor.matmul(po[:], lhsT=m[:, c, :],
                                         rhs=msg2[:, c, :],
                                         start=(ci == 0), stop=False)
                    nc.tensor.matmul(po[:], lhsT=id_t[:], rhs=g_keep[:, b, :],
                                     start=False, stop=True)
                    t4 = work.tile([P, F_out], F32, tag="t4", bufs=2)
                    nc.vector.tensor_scalar(
                        out=t4[:], in0=po[:], scalar1=dv_t[:, b:b + 1],
                        scalar2=None, op0=mybir.AluOpType.mult)
                    o_sb = work.tile([P, F_out], F32, tag="osb", bufs=2)
                    nc.vector.tensor_tensor(out=o_sb[:], in0=t4[:], in1=b2_t[:],
                                            op=mybir.AluOpType.add)
                    nc.sync.dma_start(out_r[:, b, :], o_sb[:])
                gc_base += k * c_tot
            psumC.__exit__(None, None, None)

    nc.compile()
    return nc


def _in_maps(cfg, cores, shared):
    return [{**shared, **c} for c in cores]


def _assemble(cfg, outs):
    N, F_out, npc_raw = cfg["N"], cfg["F_out"], cfg["npc_raw"]
    pos = cfg["pos"]
    full = np.empty((N, F_out), np.float32)
    for c in range(NCORES):
        nodes = np.arange(c * npc_raw, (c + 1) * npc_raw)
        full[nodes] = outs[c][pos[nodes]]
    return full


# ---------------------------------------------------------------------------
# entry points
# ---------------------------------------------------------------------------

def kernel(x, edge_index, W1, b1, W2, b2):
    cfg, cores, shared = _prep(x, edge_index, W1, b1, W2, b2)
    nc = _build_nc(cfg)
    from concourse.bass_utils import run_bass_kernel_spmd
    res = run_bass_kernel_spmd(nc, _in_maps(cfg, cores, shared),
                               list(range(NCORES)))
    return _assemble(cfg, [r["out"] for r in res.results])


def run_profiled(x, edge_index, W1, b1, W2, b2, tmpdir=None):
    """Like kernel(), but traces on HW; returns (out, exec_time_ns, tmpdir)."""
    import time

    t0 = time.time()
    cfg, cores, shared = _prep(x, edge_index, W1, b1, W2, b2)
    print(f"prep {time.time() - t0:.1f}s; cfg c_lo={cfg['c_lo']} "
          f"c_hi={cfg['c_hi']} nb={cfg['nb']}")
    t0 = time.time()
    nc = _build_nc(cfg)
    print(f"build {time.time() - t0:.1f}s; {len(nc.inst_map)} instructions")
    from concourse.bass_utils import run_bass_kernel_spmd
    in_maps = _in_maps(cfg, cores, shared)
    t0 = time.time()
    res = run_bass_kernel_spmd(nc, in_maps, list(range(NCORES)))
    print(f"run {time.time() - t0:.1f}s")
    out = _assemble(cfg, [r["out"] for r in res.results])
    exec_ns = None
    try:
        t0 = time.time()
        res2 = run_bass_kernel_spmd(nc, in_maps, list(range(NCORES)),
                                    trace=True, tmpdir=tmpdir)
        print(f"traced run {time.time() - t0:.1f}s")
        exec_ns = res2.exec_time_ns
    except Exception as e:
        print(f"trace run failed: {type(e).__name__}: {str(e)[:200]}")
    return out, exec_ns, tmpdir


def _numpy_ref(x, edge_index, W1, b1, W2, b2):
    N = x.shape[0]
    src = np.concatenate([edge_index[0], np.arange(N)])
    dst = np.concatenate([edge_index[1], np.arange(N)])
    deg = np.bincount(dst, minlength=N).astype(np.float64)
    dinv = np.where(deg > 0, 1 / np.sqrt(deg), 0)
    nrm = (dinv[src] * dinv[dst]).astype(np.float32)

    def layer(h, W, b):
        hw = h @ W
        out = np.zeros((N, W.shape[1]), np.float32)
        np.add.at(out, dst, hw[src] * nrm[:, None])
        return out + b

    h = np.maximum(layer(x, W1, b1), 0)
    return layer(h, W2, b2)


def _selftest_sim():
    from concourse import bass_interp
    rng = np.random.default_rng(1)
    N, E, F_in = 2048, 8192, 512
    x = rng.standard_normal((N, F_in), dtype=np.float32)
    ei = rng.integers(0, N, (2, E)).astype(np.int64)
    W1 = (rng.standard_normal((F_in, 256), dtype=np.float32) * F_in ** -0.5)
    W2 = (rng.standard_normal((256, 128), dtype=np.float32) * 256 ** -0.5)
    b1 = rng.standard_normal(256).astype(np.float32) * 0.1
    b2 = rng.standard_normal(128).astype(np.float32) * 0.1

    cfg, cores, shared = _prep(x, ei, W1, b1, W2, b2)
    print("cfg:", {k: v for k, v in cfg.items() if k != "pos"})
    nc = _build_nc(cfg)
    print("built; instructions:", len(nc.inst_map))

    sim = bass_interp.MultiCoreSim(nc, NCORES)
    for i, m in enumerate(_in_maps(cfg, cores, shared)):
        for k, v in m.items():
            sim.cores[i].tensor(k)[:] = v
    sim.simulate()
    outs = [np.array(sim.cores[i].mem_tensor("out")) for i in range(NCORES)]
    got = _assemble(cfg, outs)
    want = _numpy_ref(x, ei, W1, b1, W2, b2)
    err = np.abs(got - want).max() / (np.abs(want).max() + 1e-9)
    print("selftest rel err:", err)
    assert err < 1e-2, "selftest FAILED"
    print("SELFTEST PASSED")


if __name__ == "__main__":
    _selftest_sim()


# revision 16
# speedup vs baseline: 1.0048x; 1.0048x over previous
"""2-layer GCN (GCNConv x2, symmetric norm, self-loops) on 8 Trainium2 NeuronCores.

Strategy (graph/data parallel):
  - Nodes are partitioned contiguously across 8 cores (6250/core, padded to
    6272 = 49*128). Within a core, nodes are permuted into 49 blocks of 128
    with greedy in-degree balancing so every block has the same padded edge
    slot count (compile-time constant).
  - norm = dinv[src]*dinv[dst] is factored: dinv[src] is folded into x on
    the host (x' = dinv*x so h' = x'@W1 = dinv*h), dinv[dst] is applied in
    the per-block epilogue. Self-loop edges are dropped from the gather
    slots and handled by an identity matmul against the locally-kept h'/g'
    blocks (dinv_d^2*h_d = dinv_d * h'_d).
  - Layer 1 dense transform h' = x' @ W1 is row-sharded: each core
    multiplies its [6272, 4096] x'-slice (fed pre-transposed bf16) against
    replicated W1.
  - h' is AllGather'd to a full [50176, 256] per-core DRAM table; each core
    aggregates its own dst nodes: dma_gather fetches h'[src] rows and a
    0/1 one-hot selection matrix M (built on-device with ONE batched
    is_equal per superblock using stride-0 broadcast APs) scatter-adds them
    on the TensorEngine with PSUM accumulation; M is the stationary
    operand so each chunk costs one LDWEIGHTS + one 256-wide stream and
    the result lands node-major [dst, feat].
  - epilogue per block: (+ self-loop identity matmul), *dinv_d, +b1, ReLU
    (on ACT), PE-transpose, @W2, *dinv_d -> g'. g' is AllGather'd and the
    same gather/one-hot aggregation + bias produces the output.
  - dma_gather indices are int16, so the gather table is split into lo/hi
    halves (25088 rows each) and edges are slotted by src half.

kernel(**inputs) takes full unsharded inputs, returns the full [50000, 128]
output. Self-contained: no sibling imports; /opt/trn_rl_repo provides bass.
"""

import math
import sys

import numpy as np

sys.path.insert(0, "/opt/trn_rl_repo")

import concourse.bass as bass  # noqa: E402
import concourse.mybir as mybir  # noqa: E402
import concourse.tile as tile  # noqa: E402
from concourse import bacc  # noqa: E402

P = 128
NCORES = 8
SB = 2  # blocks per gather superblock
GA = 8  # blocks per phase-A psum group

F32 = mybir.dt.float32
BF16 = mybir.dt.bfloat16
I16 = mybir.dt.int16


# ---------------------------------------------------------------------------
# host-side preprocessing
# ---------------------------------------------------------------------------

def _superblocks(nb):
    return [tuple(range(s, min(s + SB, nb))) for s in range(0, nb, SB)]


def _swizzle_idx(idx):
    """gather idx j -> [j%16, j//16], replicated across the 8 groups of 16."""
    n = idx.shape[0]
    a = np.zeros((16, n // 16), np.int16)
    a[np.arange(n) % 16, np.arange(n) // 16] = idx.astype(np.int16)
    return np.tile(a, (8, 1))


def _pack_blocks_2d(dlo, dhi, nb, w_lo=1.0, w_hi=1.0):
    """Greedy pack minimizing the weighted max per-half block load (both
    halves are padded to a global per-half chunk count, so the max is what
    costs; weights normalize unequal half means)."""
    n = dlo.shape[0]
    order = np.argsort(-(dlo + dhi), kind="stable")
    cur_lo = np.zeros(nb)
    cur_hi = np.zeros(nb)
    counts = np.zeros(nb, np.int64)
    pos = np.empty(n, np.int64)
    for i in order:
        score = np.maximum((cur_lo + dlo[i]) * w_lo, (cur_hi + dhi[i]) * w_hi)
        score[counts >= P] = np.inf
        b = int(np.argmin(score))
        pos[i] = b * P + counts[b]
        counts[b] += 1
        cur_lo[b] += dlo[i]
        cur_hi[b] += dhi[i]
    return pos


def _prep(x, edge_index, W1, b1, W2, b2):
    import ml_dtypes

    N, F_in = x.shape
    F_h = W1.shape[1]
    F_out = W2.shape[1]
    assert N % NCORES == 0 and F_in % P == 0 and F_h == 2 * P and F_out == P
    npc_raw = N // NCORES
    nb = math.ceil(npc_raw / P)
    npc = nb * P
    ntot = NCORES * npc
    # lo/hi gather-table split: the boundary is the largest row count an
    # int16 index can address; the halves are unequal so the weighted
    # packer normalizes them.
    thalf = min(2 ** 15 - 1, ntot - P)
    thalf -= thalf % P

    # self-loops are NOT slotted as edges (identity-matmul path); they do
    # count toward deg.
    src = np.asarray(edge_index[0]).astype(np.int64)
    dst = np.asarray(edge_index[1]).astype(np.int64)
    deg = (np.bincount(np.concatenate([dst, np.arange(N)]), minlength=N)
           .astype(np.float64))
    dinv = np.where(deg > 0, 1.0 / np.sqrt(deg), 0.0)

    core_dst = dst // npc_raw

    # row id of src depends only on (core, pos); pos only permutes within
    # a core, so the lo/hi class of an edge is fixed by the row range the
    # src CORE occupies -- except for the boundary core, whose nodes can
    # fall on either side depending on pos. Classify provisionally by the
    # core's row base using pos-independent bounds, then pack, then
    # finalize with actual pos.
    hi_src = ((src // npc_raw) * npc) + npc // 2 >= thalf  # provisional

    # per-node in-degree by half, for block balancing
    d_lo = np.bincount(dst[~hi_src], minlength=N)
    d_hi = np.bincount(dst[hi_src], minlength=N)
    frac_lo = max(d_lo.sum(), 1) / max(d_lo.sum() + d_hi.sum(), 1)
    w_lo = 1.0 / max(frac_lo, 1e-6)
    w_hi = 1.0 / max(1.0 - frac_lo, 1e-6)

    pos = np.empty(N, np.int64)
    for c in range(NCORES):
        lo = c * npc_raw
        nodes = np.arange(lo, lo + npc_raw)
        pos[nodes] = _pack_blocks_2d(d_lo[nodes].astype(np.float64),
                                     d_hi[nodes].astype(np.float64), nb,
                                     w_lo, w_hi)
    rowof = (np.arange(N) // npc_raw) * npc + pos
    hi_src = rowof[src] >= thalf  # final, pos-exact

    # per (core, block, half) edge counts -> global chunk constants
    blk_of_dst = pos[dst] // P
    cnt = np.zeros((NCORES, nb, 2), np.int64)
    np.add.at(cnt, (core_dst, blk_of_dst, hi_src.astype(np.int64)), 1)
    c_lo = int(math.ceil(cnt[:, :, 0].max() / P))
    c_hi = int(math.ceil(cnt[:, :, 1].max() / P))
    c_tot = c_lo + c_hi

    sbs = _superblocks(nb)
    nchunks = nb * c_tot

    cores = []
    for c in range(NCORES):
        mask = core_dst == c
        e_src_row = rowof[src[mask]]
        e_hi = hi_src[mask]
        e_blk = blk_of_dst[mask]
        e_dl = (pos[dst[mask]] % P).astype(np.float32)

        # slot arrays per half: block-major, padded to C_half*128 per block
        idx_flat = {0: np.zeros(nb * c_lo * P, np.int64),
                    1: np.zeros(nb * c_hi * P, np.int64)}
        # dst-local slot value per (block, chunk, slot); 255 = inert pad
        dl_by_chunk = np.full((nb, c_tot, P), 255.0, np.float32)
        for h, c_half, base in ((0, c_lo, 0), (1, c_hi, c_lo)):
            sel = e_hi == (h == 1)
            sr = e_src_row[sel] - (thalf if h == 1 else 0)
            bl = e_blk[sel]
            order = np.argsort(bl, kind="stable")
            sr, bl = sr[order], bl[order]
            dl_s = e_dl[sel][order]
            start = np.searchsorted(bl, np.arange(nb))
            end = np.searchsorted(bl, np.arange(nb) + 1)
            for b in range(nb):
                k = end[b] - start[b]
                assert k <= c_half * P
                sl = slice(start[b], end[b])
                idx_flat[h][b * c_half * P: b * c_half * P + k] = sr[sl]
                flat_dl = np.full(c_half * P, 255.0, np.float32)
                flat_dl[:k] = dl_s[sl]
                dl_by_chunk[b, base: base + c_half] = flat_dl.reshape(c_half, P)

        assert idx_flat[0].min() >= 0 and idx_flat[0].max() < thalf
        assert idx_flat[1].min() >= 0 and idx_flat[1].max() < thalf

        # mdst in device chunk order: per superblock, [blocks x lo] then
        # [blocks x hi]; within each, chunk-major per block. Layout:
        # [P slots, nchunks] (slot value = dst-local of that slot).
        md = []
        for blocks in sbs:
            for b in blocks:
                md.append(dl_by_chunk[b, :c_lo])
            for b in blocks:
                md.append(dl_by_chunk[b, c_lo:])
        mdst = np.concatenate(md).reshape(nchunks, P).T.copy()  # [P, nchunks]

        # x slice, permuted, pre-scaled by dinv[node], transposed:
        # xt[f, pos] = dinv[node] * x[node, f]
        nodes = np.arange(c * npc_raw, (c + 1) * npc_raw)
        xp = np.zeros((npc, F_in), np.float32)
        xp[pos[nodes]] = np.asarray(x[nodes], np.float32) * \
            dinv[nodes, None].astype(np.float32)
        xt = np.ascontiguousarray(xp.T.astype(np.float32))
        xt = xt.astype(ml_dtypes.bfloat16)

        # dinv per dst slot, block-major: dinvd[p, b]
        dv = np.zeros(npc, np.float32)
        dv[pos[nodes]] = dinv[nodes].astype(np.float32)
        dinvd = dv.reshape(nb, P).T.copy()  # [P, nb]

        cores.append({
            "xt": xt,
            "idx_lo": _swizzle_idx(idx_flat[0]),
            "idx_hi": _swizzle_idx(idx_flat[1]),
            "mdst": ml_dtypes.bfloat16(mdst),
            "dinvd": dinvd,
        })

    shared = {
        "w1": ml_dtypes.bfloat16(np.asarray(W1, np.float32)),
        "w2": ml_dtypes.bfloat16(np.asarray(W2, np.float32)),
        "b1b": ml_dtypes.bfloat16(
            np.tile(np.asarray(b1, np.float32)[None, :], (P, 1))),
        "b2b": np.tile(np.asarray(b2, np.float32)[None, :], (P, 1)),
        "iota": ml_dtypes.bfloat16(
            np.tile(np.arange(P, dtype=np.float32)[None, :], (P, 1))),
        "ident": ml_dtypes.bfloat16(np.eye(P, dtype=np.float32)),
    }
    cfg = dict(N=N, F_in=F_in, F_h=F_h, F_out=F_out, npc_raw=npc_raw, nb=nb,
               npc=npc, ntot=ntot, thalf=thalf, c_lo=c_lo, c_hi=c_hi,
               c_tot=c_tot, nchunks=nchunks, pos=pos)
    return cfg, cores, shared


# ---------------------------------------------------------------------------
# device kernel
# ---------------------------------------------------------------------------

def _build_nc(cfg):
    F_in, F_h, F_out = cfg["F_in"], cfg["F_h"], cfg["F_out"]
    nb, npc, ntot, thalf = cfg["nb"], cfg["npc"], cfg["ntot"], cfg["thalf"]
    c_lo, c_hi, c_tot = cfg["c_lo"], cfg["c_hi"], cfg["c_tot"]
    nchunks = cfg["nchunks"]
    kt = F_in // P
    sbs = _superblocks(nb)
    rg = [list(range(NCORES))]

    nc = bacc.Bacc(None, num_devices=NCORES, num_swdge_queues=4)

    xt_d = nc.declare_dram_parameter("xt", [F_in, npc], BF16, isOutput=False)
    w1_d = nc.declare_dram_parameter("w1", [F_in, F_h], BF16, isOutput=False)
    w2_d = nc.declare_dram_parameter("w2", [F_h, F_out], BF16, isOutput=False)
    b1_d = nc.declare_dram_parameter("b1b", [P, F_h], BF16, isOutput=False)
    b2_d = nc.declare_dram_parameter("b2b", [P, F_out], F32, isOutput=False)
    iota_d = nc.declare_dram_parameter("iota", [P, P], BF16, isOutput=False)
    id_d = nc.declare_dram_parameter("ident", [P, P], BF16, isOutput=False)
    dv_d = nc.declare_dram_parameter("dinvd", [P, nb], F32, isOutput=False)
    ilo_d = nc.declare_dram_parameter("idx_lo", [P, nb * c_lo * 8], I16, isOutput=False)
    ihi_d = nc.declare_dram_parameter("idx_hi", [P, nb * c_hi * 8], I16, isOutput=False)
    mdst_d = nc.declare_dram_parameter("mdst", [P, nchunks], BF16, isOutput=False)
    out_d = nc.declare_dram_parameter("out", [npc, F_out], F32, isOutput=True)

    with tile.TileContext(nc) as tc:
        with (
            tc.tile_pool(name="const", bufs=1) as const,
            tc.tile_pool(name="work", bufs=1) as work,
            tc.tile_pool(name="dram", bufs=1, space="DRAM") as dram,
        ):
            h_own = dram.tile([npc, F_h], BF16)
            h_full = dram.tile([ntot, F_h], BF16, addr_space="Shared")
            g_own = dram.tile([npc, F_out], BF16)
            g_full = dram.tile([ntot, F_out], BF16, addr_space="Shared")

            w1_t = const.tile([P, kt, F_h], BF16)
            w2_t = const.tile([P, 2, F_out], BF16)
            b1_t = const.tile([P, F_h], BF16)
            b2_t = const.tile([P, F_out], F32)
            iota_t = const.tile([P, P], BF16)
            id_t = const.tile([P, P], BF16)
            dv_t = const.tile([P, nb], F32)
            ilo_t = const.tile([P, nb * c_lo * 8], I16)
            ihi_t = const.tile([P, nb * c_hi * 8], I16)
            mdst_t = const.tile([P, nchunks], BF16)
            h_keep = const.tile([P, nb, F_h], BF16)
            g_keep = const.tile([P, nb, F_out], BF16)

            w1_r = w1_d[:].rearrange("(a p) o -> p a o", p=P)
            nc.sync.dma_start(w1_t[:, 0:1, :], w1_r[:, 0:1, :])
            nc.sync.dma_start(w1_t[:, 1:, :], w1_r[:, 1:, :])
            nc.scalar.dma_start(w2_t[:], w2_d[:].rearrange("(h p) o -> p h o", p=P))
            nc.scalar.dma_start(b1_t[:], b1_d[:])
            nc.scalar.dma_start(b2_t[:], b2_d[:])
            nc.scalar.dma_start(iota_t[:], iota_d[:])
            nc.scalar.dma_start(id_t[:], id_d[:])
            nc.scalar.dma_start(dv_t[:], dv_d[:])
            nc.scalar.dma_start(ilo_t[:], ilo_d[:])
            nc.scalar.dma_start(ihi_t[:], ihi_d[:])
            nc.scalar.dma_start(mdst_t[:], mdst_d[:])

            # PE warm-up: ~40 throwaway matmuls so the HAM clock-gate is
            # released before the real phase-A stream begins.
            warm_pool = tc.tile_pool(name="psumW", bufs=1, space="PSUM")
            pw = warm_pool.__enter__()
            pwt = pw.tile([P, P], F32, tag="pw", bufs=1, space="PSUM")
            for _ in range(40):
                nc.tensor.matmul(pwt[:], lhsT=iota_t[:], rhs=iota_t[:],
                                 start=True, stop=True)
            warm_pool.__exit__(None, None, None)

            # ---- phase A: h'_own = x' @ W1 (rows = this core's nodes) ----
            # Blocks are processed in groups of GA=8 so each xt DMA reads
            # 8*128 contiguous columns (2 KB/partition); DMA issue alternates
            # between the two HWDGE rings (sync / scalar engines).
            xt_r = xt_d[:].rearrange("(a p) n -> p a n", p=P)
            h_own_r = h_own[:].rearrange("(b p) f -> p b f", p=P)
            g_own_r = g_own[:].rearrange("(b p) f -> p b f", p=P)
            out_r = out_d[:].rearrange("(b p) f -> p b f", p=P)
            psumA = tc.tile_pool(name="psumA", bufs=1, space="PSUM")
            psum = psumA.__enter__()
            dma_i = [0]

            def dma_alt(dst, src):
                eng = nc.sync if dma_i[0] % 2 == 0 else nc.scalar
                eng.dma_start(dst, src)
                dma_i[0] += 1

            for g0 in range(0, nb, GA):
                gb = list(range(g0, min(g0 + GA, nb)))
                phs = [psum.tile([P, F_h], F32, tag="ph", bufs=GA,
                                 space="PSUM", name=f"ph{g0}_{i}")
                       for i in range(len(gb))]
                for a in range(kt):
                    xt_t = work.tile([P, len(gb) * P], BF16, tag="xt", bufs=6)
                    dma_alt(xt_t[:], xt_r[:, a, g0 * P:g0 * P + len(gb) * P])
                    for i in range(len(gb)):
                        nc.tensor.matmul(phs[i][:],
                                         lhsT=xt_t[:, i * P:(i + 1) * P],
                                         rhs=w1_t[:, a, :],
                                         start=(a == 0), stop=(a == kt - 1))
                for i, b in enumerate(gb):
                    nc.vector.tensor_copy(h_keep[:, b, :], phs[i][:])
                dma_alt(h_own_r[:, g0:g0 + len(gb), :],
                        h_keep[:, g0:g0 + len(gb), :])

            psumA.__exit__(None, None, None)
            nc.gpsimd.collective_compute(
                "AllGather", mybir.AluOpType.bypass, replica_groups=rg,
                ins=[h_own[:]], outs=[h_full[:]],
            )
            psumC = tc.tile_pool(name="psumC", bufs=1, space="PSUM")
            psum = psumC.__enter__()

            # HW cap: >768 idxs in one dma_gather crashes the exec unit
            # (~128 in-flight descs per Q7 core); split and spread over the
            # 4 SWDGE queues.
            GMAX = 6  # chunks (of 128 idxs) per dma_gather op
            qn = [0]

            def gathers(dst, c0, nch, table, idx_t, col0, elem):
                for s in range(0, nch, GMAX):
                    k = min(GMAX, nch - s)
                    nc.gpsimd.dma_gather(
                        out_ap=dst[:, c0 + s:c0 + s + k, :], in_ap=table,
                        idxs_ap=idx_t[:, col0 + s * 8:col0 + (s + k) * 8],
                        num_idxs=k * P, num_idxs_reg=k * P, elem_size=elem,
                        queue_num=qn[0] % 4)
                    qn[0] += 1

            def m_build(gc0, nch):
                m = work.tile([P, nch, P], BF16, tag="m", bufs=3)
                nc.vector.tensor_tensor(
                    out=m[:],
                    in0=iota_t[:].unsqueeze(1).to_broadcast([P, nch, P]),
                    in1=mdst_t[:, gc0:gc0 + nch].unsqueeze(2)
                        .to_broadcast([P, nch, P]),
                    op=mybir.AluOpType.is_equal)
                return m

            # ---- phase C: aggregate layer 1, epilogue, transform by W2 ----
            gc_base = 0
            for blocks in sbs:
                k = len(blocks)
                msg = work.tile([P, k * c_tot, F_h], BF16, tag="msg", bufs=4)
                b0 = blocks[0]
                gathers(msg, 0, k * c_lo, h_full[0:thalf, :],
                        ilo_t, b0 * c_lo * 8, F_h)
                gathers(msg, k * c_lo, k * c_hi, h_full[thalf:ntot, :],
                        ihi_t, b0 * c_hi * 8, F_h)
                m = m_build(gc_base, k * c_tot)
                for bi, b in enumerate(blocks):
                    pa = psum.tile([P, F_h], F32, tag="pa", bufs=2, space="PSUM")
                    chunks = ([bi * c_lo + j for j in range(c_lo)] +
                              [k * c_lo + bi * c_hi + j for j in range(c_hi)])
                    for ci, c in enumerate(chunks):
                        nc.tensor.matmul(pa[:], lhsT=m[:, c, :],
                                         rhs=msg[:, c, :],
                                         start=(ci == 0), stop=False)
                    # self-loop: += I * h'_block (dinv_d^2 h = dinv_d h')
                    nc.tensor.matmul(pa[:], lhsT=id_t[:], rhs=h_keep[:, b, :],
                                     start=False, stop=True)
                    t2 = work.tile([P, F_h], BF16, tag="t2", bufs=2)
                    nc.vector.tensor_scalar(
                        out=t2[:], in0=pa[:], scalar1=dv_t[:, b:b + 1],
                        scalar2=None, op0=mybir.AluOpType.mult)
                    t3 = work.tile([P, F_h], BF16, tag="t3", bufs=2)
                    nc.vector.tensor_tensor(out=t3[:], in0=t2[:], in1=b1_t[:],
                                            op=mybir.AluOpType.add)
                    rh = work.tile([P, F_h], BF16, tag="rh", bufs=2)
                    nc.scalar.activation(rh[:], t3[:],
                                         mybir.ActivationFunctionType.Relu)
                    pg = psum.tile([P, F_out], F32, tag="pg", bufs=2,
                                   space="PSUM")
                    for half in range(2):
                        pt = psum.tile([P, P], BF16, tag="pt", bufs=2,
                                       space="PSUM")
                        nc.tensor.transpose(
                            pt[:], rh[:, half * P:(half + 1) * P], id_t[:])
                        rt = work.tile([P, P], BF16, tag="rt", bufs=2)
                        nc.vector.tensor_copy(rt[:], pt[:])
                        nc.tensor.matmul(pg[:], lhsT=rt[:],
                                         rhs=w2_t[:, half, :],
                                         start=(half == 0), stop=(half == 1))
                    # g' = dinv_d * g
                    nc.vector.tensor_scalar(
                        out=g_keep[:, b, :], in0=pg[:],
                        scalar1=dv_t[:, b:b + 1], scalar2=None,
                        op0=mybir.AluOpType.mult)
                dma_alt(g_own_r[:, b0:b0 + k, :], g_keep[:, b0:b0 + k, :])
                gc_base += k * c_tot

            nc.gpsimd.collective_compute(
                "AllGather", mybir.AluOpType.bypass, replica_groups=rg,
                ins=[g_own[:]], outs=[g_full[:]],
            )

            # ---- phase E: aggregate layer 2, add bias, write out ----
            gc_base = 0
            for blocks in sbs:
                k = len(blocks)
                msg2 = work.tile([P, k * c_tot, F_out], BF16, tag="msg", bufs=4)
                b0 = blocks[0]
                gathers(msg2, 0, k * c_lo, g_full[0:thalf, :],
                        ilo_t, b0 * c_lo * 8, F_out)
                gathers(msg2, k * c_lo, k * c_hi, g_full[thalf:ntot, :],
                        ihi_t, b0 * c_hi * 8, F_out)
                m = m_build(gc_base, k * c_tot)
                for bi, b in enumerate(blocks):
                    po = psum.tile([P, F_out], F32, tag="pa", bufs=2,
                                   space="PSUM")
                    chunks = ([bi * c_lo + j for j in range(c_lo)] +
                              [k * c_lo + bi * c_hi + j for j in range(c_hi)])
                    for ci, c in enumerate(chunks):
                        nc.tensor.matmul(po[:], lhsT=m[:, c, :],
                                         rhs=msg2[:, c, :],
                                         start=(ci == 0), stop=False)
                    nc.tensor.matmul(po[:], lhsT=id_t[:], rhs=g_keep[:, b, :],
                                     start=False, stop=True)
                    t4 = work.tile([P, F_out], F32, tag="t4", bufs=2)
                    nc.vector.tensor_scalar(
                        out=t4[:], in0=po[:], scalar1=dv_t[:, b:b + 1],
                        scalar2=None, op0=mybir.AluOpType.mult)
                    o_sb = work.tile([P, F_out], F32, tag="osb", bufs=2)
                    nc.vector.tensor_tensor(out=o_sb[:], in0=t4[:], in1=b2_t[:],
                                            op=mybir.AluOpType.add)
                    nc.sync.dma_start(out_r[:, b, :], o_sb[:])
                gc_base += k * c_tot
            psumC.__exit__(None, None, None)

    nc.compile()
    return nc


def _in_maps(cfg, cores, shared):
    return [{**shared, **c} for c in cores]


def _assemble(cfg, outs):
    N, F_out, npc_raw = cfg["N"], cfg["F_out"], cfg["npc_raw"]
    pos = cfg["pos"]
    full = np.empty((N, F_out), np.float32)
    for c in range(NCORES):
        nodes = np.arange(c * npc_raw, (c + 1) * npc_raw)
        full[nodes] = outs[c][pos[nodes]]
    return full


# ---------------------------------------------------------------------------
# entry points
# ---------------------------------------------------------------------------

def kernel(x, edge_index, W1, b1, W2, b2):
    cfg, cores, shared = _prep(x, edge_index, W1, b1, W2, b2)
    nc = _build_nc(cfg)
    from concourse.bass_utils import run_bass_kernel_spmd
    res = run_bass_kernel_spmd(nc, _in_maps(cfg, cores, shared),
                               list(range(NCORES)))
    return _assemble(cfg, [r["out"] for r in res.results])


def run_profiled(x, edge_index, W1, b1, W2, b2, tmpdir=None):
    """Like kernel(), but traces on HW; returns (out, exec_time_ns, tmpdir)."""
    import time

    t0 = time.time()
    cfg, cores, shared = _prep(x, edge_index, W1, b1, W2, b2)
    print(f"prep {time.time() - t0:.1f}s; cfg c_lo={cfg['c_lo']} "
          f"c_hi={cfg['c_hi']} nb={cfg['nb']}")
    t0 = time.time()
    nc = _build_nc(cfg)
    print(f"build {time.time() - t0:.1f}s; {len(nc.inst_map)} instructions")
    from concourse.bass_utils import run_bass_kernel_spmd
    in_maps = _in_maps(cfg, cores, shared)
    t0 = time.time()
    res = run_bass_kernel_spmd(nc, in_maps, list(range(NCORES)))
    print(f"run {time.time() - t0:.1f}s")
    out = _assemble(cfg, [r["out"] for r in res.results])
    exec_ns = None
    try:
        t0 = time.time()
        res2 = run_bass_kernel_spmd(nc, in_maps, list(range(NCORES)),
                                    trace=True, tmpdir=tmpdir)
        print(f"traced run {time.time() - t0:.1f}s")
        exec_ns = res2.exec_time_ns
    except Exception as e:
        print(f"trace run failed: {type(e).__name__}: {str(e)[:200]}")
    return out, exec_ns, tmpdir


def _numpy_ref(x, edge_index, W1, b1, W2, b2):
    N = x.shape[0]
    src = np.concatenate([edge_index[0], np.arange(N)])
    dst = np.concatenate([edge_index[1], np.arange(N)])
    deg = np.bincount(dst, minlength=N).astype(np.float64)
    dinv = np.where(deg > 0, 1 / np.sqrt(deg), 0)
    nrm = (dinv[src] * dinv[dst]).astype(np.float32)

    def layer(h, W, b):
        hw = h @ W
        out = np.zeros((N, W.shape[1]), np.float32)
        np.add.at(out, dst, hw[src] * nrm[:, None])
        return out + b

    h = np.maximum(layer(x, W1, b1), 0)
    return layer(h, W2, b2)


def _selftest_sim():
    from concourse import bass_interp
    rng = np.random.default_rng(1)
    N, E, F_in = 2048, 8192, 512
    x = rng.standard_normal((N, F_in), dtype=np.float32)
    ei = rng.integers(0, N, (2, E)).astype(np.int64)
    W1 = (rng.standard_normal((F_in, 256), dtype=np.float32) * F_in ** -0.5)
    W2 = (rng.standard_normal((256, 128), dtype=np.float32) * 256 ** -0.5)
    b1 = rng.standard_normal(256).astype(np.float32) * 0.1
    b2 = rng.standard_normal(128).astype(np.float32) * 0.1

    cfg, cores, shared = _prep(x, ei, W1, b1, W2, b2)
    print("cfg:", {k: v for k, v in cfg.items() if k != "pos"})
    nc = _build_nc(cfg)
    print("built; instructions:", len(nc.inst_map))

    sim = bass_interp.MultiCoreSim(nc, NCORES)
    for i, m in enumerate(_in_maps(cfg, cores, shared)):
        for k, v in m.items():
            sim.cores[i].tensor(k)[:] = v
    sim.simulate()
    outs = [np.array(sim.cores[i].mem_tensor("out")) for i in range(NCORES)]
    got = _assemble(cfg, outs)
    want = _numpy_ref(x, ei, W1, b1, W2, b2)
    err = np.abs(got - want).max() / (np.abs(want).max() + 1e-9)
    print("selftest rel err:", err)
    assert err < 1e-2, "selftest FAILED"
    print("SELFTEST PASSED")


if __name__ == "__main__":
    _selftest_sim()
